# revision 34
# baseline (speedup 1.0000x reference)
"""Masked cross-attention + linear_in/linear_out, fused Trainium2 kernel (v2).

Problem (nn_Attention_50096498541174):
    q_proj = query @ W_in.T                         [B,T,H]
    score  = q_proj @ enc.T  (masked by src_lengths)[B,T,S]
    p      = softmax(score, -1)
    c      = p @ enc                                [B,T,H]
    out    = tanh(concat(query, c) @ W_out.T + b)   [B,T,H]

Sharding: data-parallel over batch B=32 across 8 NeuronCores (4 slots/core),
weights replicated, no collectives.  Batches are sorted by src_length (desc)
and dealt round-robin to cores; each slot is traced for the max padded length
over the cores sharing it (same program per core -> one SPMD NEFF).

v2 layout/dtype plan (vs the f32r baseline):
  - S1 (q_proj) stays f32r for accuracy: score errors are the dominant
    error term and qp feeds the scores.  Everything downstream is bf16
    (enc, qp, p, c, W_out), halving the big DMA transfers and running all
    PE transposes at 1 cycle/row.  Expected rel err ~1e-2 (gate 2e-2).
  - Lengths padded to 32 (not 128): S2/S3/enc-transposes shrink ~30% on
    average; partial 128-blocks handled with partial-partition matmuls.
  - One PSUM pool, 8 banks hand-assigned via tags; one SBUF data pool.
    (Each tile pool costs a ~0.7us teardown barrier round at kernel end.)
  - Software-pipelined slot loop; per-engine queue order chosen so the
    softmax latency chain (DVE max -> ACT exp -> DVE recip) and all PSUM
    evictions hide under independent PE work from the adjacent slots:
      PE:  pT(b) S3(b) cT(b) encNtr(b+1) S2(b+1) S4suf(b) S4pre(b+1)
      DVE: pT-ev c-ev cT-ev encN-ev rmax recip   (emission order)
      ACT: exp(b+1) tanh(b)
  - query is DMA'd twice (f32r for S1, bf16 for S4) instead of casting
    on-chip; W_out/enc/maskbias/ones/bias are bf16 host-side.
"""

import os

import numpy as np

import concourse.bass as bass
import concourse.mybir as mybir
import concourse.tile as tile
from concourse import bacc
from concourse.bass_utils import run_bass_kernel_spmd
from concourse.masks import make_identity

# Problem shape (hardcoded per the harness contract).
B, T, S, H = 32, 128, 512, 1024
NCORES = 8
NB = B // NCORES          # batches (slots) per core
TB = NB * T               # stacked query rows per core (512)
K2 = 2 * H
NEG = np.float32(-1e9)

P = 128                   # SBUF/PSUM partitions
KH = H // P               # 8 k-tiles over H
NHALF = H // 512          # 2 PSUM-bank halves of H

F32 = mybir.dt.float32
F32R = mybir.dt.float32r
BF16 = mybir.dt.bfloat16

PAD = int(os.environ.get("KERNEL_PAD", "32"))
S1_BF16 = os.environ.get("KERNEL_S1_BF16", "1") == "1"
WARMUP_MMS = int(os.environ.get("KERNEL_WARMUP_MMS", "8"))
ENC_DMA_FROM = int(os.environ.get("KERNEL_ENC_DMA_FROM", "1"))
XBAR_PT = os.environ.get("KERNEL_XBAR_PT", "1") == "1"

S1_DT = BF16 if S1_BF16 else F32R
NP_S1 = np.dtype(mybir.dt.np(S1_DT))
NP_BF16 = np.dtype(mybir.dt.np(BF16))


def _slot_plan(lens):
    """Sort batches by length (desc), deal round-robin to cores.

    Returns (order, slot_lens): order[j*NCORES + c] is the original batch
    index placed on core c, slot j; slot_lens[j] is the padded source length
    traced for slot j (max over the cores sharing that slot).
    """
    lens = np.asarray(lens, dtype=np.int64)
    order = np.argsort(-lens, kind="stable")
    pad = np.clip(np.ceil(lens[order] / PAD).astype(np.int64) * PAD, PAD, S)
    slot_lens = tuple(
        int(pad[j * NCORES : (j + 1) * NCORES].max()) for j in range(NB)
    )
    return order, slot_lens


def _emit(nc, tc, slot_lens):
    X = mybir.AxisListType
    AF = mybir.ActivationFunctionType
    ts = bass.ts

    qT_d = nc.dram_tensor("qT", [H, TB], S1_DT, kind="ExternalInput").ap()
    qTb_d = nc.dram_tensor("qTb", [H, TB], BF16, kind="ExternalInput").ap()
    winT_d = nc.dram_tensor("winT", [H, H], S1_DT, kind="ExternalInput").ap()
    woutT_d = nc.dram_tensor("woutT", [K2, H], BF16, kind="ExternalInput").ap()
    encT_d = [
        nc.dram_tensor(f"encT{b}", [H, slot_lens[b]], BF16, kind="ExternalInput").ap()
        for b in range(NB)
    ]
    # natural-layout enc for slots >= ENC_DMA_FROM (S3 rhs comes straight
    # from DRAM instead of PE transposes; slot 0's window is DMA-tight so it
    # keeps the on-chip transpose path)
    encN_d = [
        nc.dram_tensor(f"encN{b}", [slot_lens[b], H], BF16, kind="ExternalInput").ap()
        if b >= ENC_DMA_FROM else None
        for b in range(NB)
    ]
    mb_d = nc.dram_tensor("maskbias", [NB, S], BF16, kind="ExternalInput").ap()
    bias_d = nc.dram_tensor("bias", [H], BF16, kind="ExternalInput").ap()
    ones_d = nc.dram_tensor("ones", [P], BF16, kind="ExternalInput").ap()
    out_d = nc.dram_tensor("out", [NB, T, H], F32, kind="ExternalOutput").ap()

    # per-slot geometry: number of 128-chunks and width of the last chunk
    KS = [(L + P - 1) // P for L in slot_lens]
    REM = [L - P * (k - 1) for L, k in zip(slot_lens, KS)]

    with (
        tc.tile_pool(name="sb", bufs=1) as sb,
        tc.tile_pool(name="ps", bufs=1, space="PSUM") as ps,
    ):
        # ---- persistent SBUF tiles ----
        qT_sb = sb.tile([P, KH, TB], S1_DT, tag="qT")
        qpT_sb = sb.tile([P, KH, TB], BF16, tag="qpT")
        qTb_sb = sb.tile([P, KH, TB], BF16, tag="qTb")
        w_out_sb = sb.tile([P, 2 * KH, H], BF16, tag="wout")
        ones_sb = sb.tile([1, P], BF16, tag="ones")
        bias_sb = sb.tile([1, H], BF16, tag="bias")
        mb_sb = sb.tile([1, NB, S], BF16, tag="mb")
        idr_sb = sb.tile([P, P], BF16, tag="idr")
        scratch = sb.tile([P, 512], BF16, tag="scratch")

        # PSUM: one pool, 8 banks via same-size tags ([P,512] f32 = 2KB).
        PS_TAGS = [f"b{i}" for i in range(8)]

        def ps_tile(i, name, dt=F32, shape=(P, 512)):
            return ps.tile(list(shape), dt, tag=PS_TAGS[i], name=name,
                           padded_shape=[P, 512] if dt == F32 else [P, 1024])

        # warmup scratch memset is quick; warmup matmuls release the PE HAM
        # clock gate while the first DMAs stream in.  Small input DMAs go on
        # the (otherwise idle) gpsimd queue to keep sync free for the big
        # streaming loads.
        nc.gpsimd.memset(scratch[:].bitcast(F32), 0.0)
        make_identity(nc, idr_sb[:])
        nc.scalar.dma_start(out=ones_sb[:], in_=ones_d[None, :])
        nc.scalar.dma_start(out=mb_sb[:], in_=mb_d[None, :, :])
        nc.scalar.dma_start(out=bias_sb[:], in_=bias_d[None, :])

        # ---- S1: q_projT = W_in^T-blocks @ qT, kh-outer over all 8 banks.
        qTb_r = qTb_d.rearrange("(kh p) t -> p kh t", p=P)
        woutT_r = woutT_d.rearrange("(g kk p) h -> g p kk h", p=P, g=4)

        qp_ps = [ps_tile(mg, f"qp_ps{mg}") for mg in range(KH)]
        if WARMUP_MMS:
            with nc.named_scope("warmup"):
                for _ in range(WARMUP_MMS):
                    nc.tensor.matmul(
                        qp_ps[7][:], scratch[:, 0:P], scratch[:],
                        start=True, stop=True, skip_group_check=True,
                    )

        def _load_encT(j):
            t = sb.tile([P, KH, slot_lens[j]], BF16, tag="encT", bufs=3,
                        name=f"encT_sb{j}")
            nc.sync.dma_start(
                out=t[:], in_=encT_d[j].rearrange("(kh p) s -> p kh s", p=P)
            )
            return t

        encT_sb = [None] * NB
        encN_sb = [None] * NB
        qT_rk = qT_d.rearrange("(kh p) t -> p kh t", p=P)
        winT_rp = winT_d.rearrange("(kh p) g -> p kh g", p=P)

        def _load_encN(j):
            """Natural-layout enc rows straight from DRAM (gpsimd queue)."""
            t = sb.tile([P, KS[j], H], BF16, tag="encN", bufs=3,
                        name=f"encN_sb{j}")
            encN_sb[j] = t
            nfull = KS[j] - 1
            if nfull:
                nc.sync.dma_start(
                    out=t[:, 0:nfull, :],
                    in_=encN_d[j][0 : nfull * P, :].rearrange(
                        "(ks p) h -> p ks h", p=P
                    ),
                )
            nc.sync.dma_start(
                out=t[0 : REM[j], nfull, :], in_=encN_d[j][nfull * P :, :]
            )

        with nc.named_scope("s1"):
            # S1 streaming loads: fine-grained first chunks so the first
            # matmul group starts ASAP, coarser after.  Everything else is
            # dispatched behind them in first-use order.
            WCH = [(0, 1), (1, 2), (2, 4), (4, 6), (6, 8)]
            w_t = {}
            nc.sync.dma_start(out=qT_sb[:, 0:1, :], in_=qT_rk[:, 0:1, :])
            for i, (lo, hi) in enumerate(WCH):
                t = sb.tile([P, hi - lo, H], S1_DT, tag="win", bufs=5,
                            name=f"w_t{i}")
                for kh in range(lo, hi):
                    w_t[kh] = (t, kh - lo)
                nc.sync.dma_start(out=t[:], in_=winT_rp[:, lo:hi, :])
                if i == 0:
                    nc.sync.dma_start(out=qT_sb[:, 1:4, :], in_=qT_rk[:, 1:4, :])
                elif i == 2:
                    nc.sync.dma_start(out=qT_sb[:, 4:8, :], in_=qT_rk[:, 4:8, :])
            encT_sb[0] = _load_encT(0)
            encT_sb[1] = _load_encT(1)
            nc.sync.dma_start(out=qTb_sb[:], in_=qTb_r[:])
            for g in range(4):
                nc.sync.dma_start(
                    out=w_out_sb[:, 4 * g : 4 * g + 4, :], in_=woutT_r[g]
                )
            for j in range(ENC_DMA_FROM, 2):
                _load_encN(j)
            for kh in range(KH):
                wt, wi = w_t[kh]
                for mg in range(KH):
                    nc.tensor.matmul(
                        qp_ps[mg][:],
                        wt[:, wi, ts(mg, P)],
                        qT_sb[:, kh, :],
                        start=(kh == 0),
                        stop=(kh == KH - 1),
                    )
                    if kh == KH - 1:
                        nc.vector.tensor_copy(qpT_sb[:, mg, :], qp_ps[mg][:])

        # ---- slot-loop state ----
        # tr-bank rotation for transpose staging (banks 6, 7)
        tr_state = [6]

        def tr_tile(name, dt=BF16, shape=(P, 4, P)):
            i = tr_state[0]
            tr_state[0] = 13 - i  # 6 <-> 7
            return ps.tile(list(shape), dt, tag=PS_TAGS[i], name=name,
                           padded_shape=[P, 4, P])

        p_sb = [None] * NB
        pT_sb = [None] * NB
        rinv = [None] * NB
        c_sb = [None] * NB
        cT_sb = [None] * NB
        o_ps = [None] * NB

        def emit_encNtr(j):
            """encN[j] (natural [s,h] tiles) from encT[j] via PE transposes."""
            t = sb.tile([P, KS[j], H], BF16, tag="encN", bufs=3,
                        name=f"encN_sb{j}")
            encN_sb[j] = t
            for ks in range(KS[j]):
                cw = REM[j] if ks == KS[j] - 1 else P
                for half in range(2):
                    e_ps = tr_tile(f"encNtr{j}_{ks}_{half}")
                    for i in range(4):
                        ih = half * 4 + i
                        nc.tensor.transpose(
                            e_ps[0:cw, i, :],
                            encT_sb[j][:, ih, ks * P : ks * P + cw],
                            idr_sb[:],
                        )
                    nc.vector.tensor_copy(
                        t[0:cw, ks, half * 512 : half * 512 + 512],
                        e_ps[0:cw, :, :],
                    )

        def emit_s2(j):
            """score(j) + additive length mask into sm bank (j%2)."""
            Ln = slot_lens[j]
            sc = ps_tile(j % 2, f"score{j}", shape=(P, Ln))
            for kh in range(KH):
                nc.tensor.matmul(
                    sc[:],
                    qpT_sb[:, kh, ts(j, T)],
                    encT_sb[j][:, kh, :],
                    start=(kh == 0),
                    stop=False,
                )
            nc.tensor.matmul(
                sc[:], ones_sb[:], mb_sb[:, j, 0:Ln], start=False, stop=True
            )
            return sc

        def emit_softmax(j, sc):
            Ln = slot_lens[j]
            negmax = sb.tile([P, 1], F32, tag="negmax", bufs=2, name="negmax")
            nc.vector.reduce_max(negmax[:], sc[:], axis=X.X, negate=True)
            p_sb[j] = sb.tile([P, KS[j] * P], BF16, tag="p", bufs=2,
                              name=f"p{j}")
            rowsum = sb.tile([P, 1], F32, tag="rowsum", bufs=2, name="rowsum")
            nc.scalar.activation(
                p_sb[j][:, 0:Ln], sc[:], AF.Exp, bias=negmax[:],
                accum_out=rowsum[:],
            )
            rinvb = sb.tile([P, 1], F32, tag="rinv", bufs=2, name=f"rinv{j}")
            nc.vector.reciprocal(rinvb[:], rowsum[:])
            # diag(1/rowsum): used as the cT-transpose "identity" so the
            # softmax normalization rides along for free.
            rinv[j] = sb.tile([P, P], BF16, tag="diag", bufs=2,
                              name=f"diag{j}")
            nc.vector.tensor_scalar_mul(rinv[j][:], idr_sb[:], rinvb[:])
            # p -> pT chunks on the XBAR (DMA) engines; runs during the
            # previous slot's suffix, entirely off the PE.
            pT_sb[j] = sb.tile([P, KS[j], P], BF16, tag="pT", bufs=2,
                               name=f"pT{j}")
            if XBAR_PT:
                for ks in range(KS[j]):
                    nc.sync.dma_start_transpose(
                        pT_sb[j][:, ks, :], p_sb[j][:, ks * P : (ks + 1) * P]
                    )
            else:
                pT_ps = tr_tile(f"pTtr{j}")
                for ks in range(KS[j]):
                    nc.tensor.transpose(
                        pT_ps[:, ks, :], p_sb[j][:, ks * P : (ks + 1) * P],
                        idr_sb[:],
                    )
                nc.vector.tensor_copy(pT_sb[j][:], pT_ps[:, 0 : KS[j], :])

        def emit_prefix(j):
            """S4 query-half: [q] @ W_out_q into a-banks (2, 3); no stop."""
            o_ps[j] = []
            for nh in range(NHALF):
                o = ps_tile(2 + nh, f"o_ps{j}_{nh}")
                o_ps[j].append(o)
                for kk in range(KH):
                    nc.tensor.matmul(
                        o[:],
                        qTb_sb[:, kk, ts(j, T)],
                        w_out_sb[:, kk, ts(nh, 512)],
                        start=(kk == 0),
                        stop=False,
                    )

        def emit_s3(j):
            """c~ = pT^T @ encN into c banks (4, 5); plain-copy evict (the
            1/rowsum normalization is folded into the cT transpose)."""
            c_sb[j] = sb.tile([P, H], BF16, tag="c", bufs=2, name=f"c{j}")
            for nh in range(NHALF):
                c_ps = ps_tile(4 + nh, f"c_ps{j}_{nh}")
                for ks in range(KS[j]):
                    cw = REM[j] if ks == KS[j] - 1 else P
                    nc.tensor.matmul(
                        c_ps[:],
                        pT_sb[j][0:cw, ks, :],
                        encN_sb[j][0:cw, ks, ts(nh, 512)],
                        start=(ks == 0),
                        stop=(ks == KS[j] - 1),
                    )
                nc.vector.tensor_copy(c_sb[j][:, ts(nh, 512)], c_ps[:])

        def emit_cT(j):
            """c~ -> cT via regular matmuls against diag(1/rowsum): the PE
            transpose mode ignores its rhs, but c~^T @ diag(rinv) as a plain
            matmul transposes AND normalizes in one pass."""
            cT_sb[j] = sb.tile([P, KH, P], BF16, tag="cT", bufs=2,
                               name=f"cT{j}")
            for half in range(2):
                cT_ps = tr_tile(f"cTtr{j}_{half}", dt=F32)
                for i in range(4):
                    nc.tensor.matmul(
                        cT_ps[:, i, :], c_sb[j][:, ts(half * 4 + i, P)],
                        rinv[j][:], start=True, stop=True,
                    )
                nc.vector.tensor_copy(
                    cT_sb[j][:, half * 4 : half * 4 + 4, :], cT_ps[:]
                )

        def emit_suffix(j):
            """S4 context-half + bias; tanh-evict and store per nh half."""
            out_sb = sb.tile([P, H], F32, tag="out", bufs=2, name=f"out{j}")
            for nh in range(NHALF):
                nsl = ts(nh, 512)
                for kk in range(KH):
                    nc.tensor.matmul(
                        o_ps[j][nh][:],
                        cT_sb[j][:, kk, :],
                        w_out_sb[:, KH + kk, nsl],
                        start=False,
                        stop=False,
                    )
                nc.tensor.matmul(
                    o_ps[j][nh][:], ones_sb[:], bias_sb[:, nsl],
                    start=False, stop=True,
                )
                nc.scalar.activation(out_sb[:, nsl], o_ps[j][nh][:], AF.Tanh)
                nc.sync.dma_start(out=out_d[j][:, nsl], in_=out_sb[:, nsl])

        # ---- prologue: slot 0 head; prefix(0) covers softmax(0) latency.
        with nc.named_scope("b0h"):
            if ENC_DMA_FROM > 0:
                emit_encNtr(0)
            sc0 = emit_s2(0)
            emit_softmax(0, sc0)
            emit_prefix(0)

        # ---- software-pipelined slot loop ----
        for b in range(NB):
            scope = nc.named_scope(f"b{b}")
            scope.__enter__()
            emit_s3(b)
            emit_cT(b)
            if b + 1 < NB:
                if b + 2 < NB:
                    encT_sb[b + 2] = _load_encT(b + 2)
                    if b + 2 >= ENC_DMA_FROM:
                        _load_encN(b + 2)
                if b + 1 < ENC_DMA_FROM:
                    emit_encNtr(b + 1)
                sc = emit_s2(b + 1)
                emit_softmax(b + 1, sc)
            emit_suffix(b)
            if b + 1 < NB:
                emit_prefix(b + 1)
            scope.__exit__(None, None, None)


def build_nc(slot_lens=(S,) * NB):
    # Bacc (not raw Bass): its lowering splits multi-sem waits and moves
    # matmul waits onto ldweights, which TRN2 codegen requires.
    nc = bacc.Bacc("TRN2", target_bir_lowering=False, debug=False)
    with tile.TileContext(nc) as tc:
        _emit(nc, tc, slot_lens)
    nc.compile()
    return nc


_NC_CACHE = {}


def _get_nc(slot_lens):
    key = (S1_DT, PAD, ENC_DMA_FROM, slot_lens)
    if key not in _NC_CACHE:
        _NC_CACHE[key] = build_nc(slot_lens)
    return _NC_CACHE[key]


def make_in_maps(query, encoder_outputs, src_lengths, W_in, W_out, b_out):
    """Host-side sharding + layout prep.

    Returns (in_maps, order, slot_lens): one input map per core; order maps
    (slot j, core c) -> original batch index order[j*NCORES + c].
    """
    query = np.asarray(query, dtype=np.float32)
    enc = np.asarray(encoder_outputs, dtype=np.float32)
    lens = np.asarray(src_lengths, dtype=np.int32)
    order, slot_lens = _slot_plan(lens)

    w_inT = np.ascontiguousarray(np.asarray(W_in, dtype=np.float32).T).astype(NP_S1)
    w_outT = np.ascontiguousarray(np.asarray(W_out, dtype=np.float32).T).astype(NP_BF16)
    bias = np.asarray(b_out, dtype=np.float32).astype(NP_BF16)
    ones = np.ones((P,), dtype=NP_BF16)

    in_maps = []
    for c in range(NCORES):
        idx = [int(order[j * NCORES + c]) for j in range(NB)]
        q_c = query[idx]                      # [NB, T, H] in slot order
        qT = np.ascontiguousarray(q_c.transpose(2, 0, 1)).reshape(H, TB)
        maskbias = np.where(
            np.arange(S, dtype=np.int64)[None, :]
            < lens[idx][:, None].astype(np.int64),
            np.float32(0.0),
            NEG,
        ).astype(NP_BF16)
        im = {
            "qT": qT.astype(NP_S1),
            "qTb": qT.astype(NP_BF16),
            "winT": w_inT,
            "woutT": w_outT,
            "maskbias": maskbias,
            "bias": bias,
            "ones": ones,
        }
        for j in range(NB):
            Ln = slot_lens[j]
            e_b = enc[idx[j], :Ln, :]         # [Ln, H]
            im[f"encT{j}"] = np.ascontiguousarray(e_b.T).astype(NP_BF16)
            if j >= ENC_DMA_FROM:
                im[f"encN{j}"] = np.ascontiguousarray(e_b).astype(NP_BF16)
        in_maps.append(im)
    return in_maps, order, slot_lens


def run(query, encoder_outputs, src_lengths, W_in, W_out, b_out, **spmd_kwargs):
    in_maps, order, slot_lens = make_in_maps(
        query, encoder_outputs, src_lengths, W_in, W_out, b_out
    )
    res = run_bass_kernel_spmd(
        _get_nc(slot_lens), in_maps, list(range(NCORES)), **spmd_kwargs
    )
    out = np.empty((B, T, H), dtype=np.float32)
    for c in range(NCORES):
        core_out = res.results[c]["out"]      # [NB, T, H] in slot order
        for j in range(NB):
            out[int(order[j * NCORES + c])] = core_out[j]
    return out, res


def kernel(query, encoder_outputs, src_lengths, W_in, W_out, b_out):
    out, _ = run(query, encoder_outputs, src_lengths, W_in, W_out, b_out)
    return out


# revision 40
# speedup vs baseline: 1.2101x; 1.2101x over previous
"""Masked cross-attention + linear_in/linear_out, fused Trainium2 kernel (v2).

Problem (nn_Attention_50096498541174):
    q_proj = query @ W_in.T                         [B,T,H]
    score  = q_proj @ enc.T  (masked by src_lengths)[B,T,S]
    p      = softmax(score, -1)
    c      = p @ enc                                [B,T,H]
    out    = tanh(concat(query, c) @ W_out.T + b)   [B,T,H]

Sharding: data-parallel over batch B=32 across 8 NeuronCores (4 slots/core),
weights replicated, no collectives.  Batches are sorted by src_length (desc)
and dealt round-robin to cores; each slot is traced for the max padded length
over the cores sharing it (same program per core -> one SPMD NEFF).

v2 layout/dtype plan (vs the f32r baseline):
  - S1 (q_proj) stays f32r for accuracy: score errors are the dominant
    error term and qp feeds the scores.  Everything downstream is bf16
    (enc, qp, p, c, W_out), halving the big DMA transfers and running all
    PE transposes at 1 cycle/row.  Expected rel err ~1e-2 (gate 2e-2).
  - Lengths padded to 32 (not 128): S2/S3/enc-transposes shrink ~30% on
    average; partial 128-blocks handled with partial-partition matmuls.
  - One PSUM pool, 8 banks hand-assigned via tags; one SBUF data pool.
    (Each tile pool costs a ~0.7us teardown barrier round at kernel end.)
  - Software-pipelined slot loop; per-engine queue order chosen so the
    softmax latency chain (DVE max -> ACT exp -> DVE recip) and all PSUM
    evictions hide under independent PE work from the adjacent slots:
      PE:  pT(b) S3(b) cT(b) encNtr(b+1) S2(b+1) S4suf(b) S4pre(b+1)
      DVE: pT-ev c-ev cT-ev encN-ev rmax recip   (emission order)
      ACT: exp(b+1) tanh(b)
  - query is DMA'd twice (f32r for S1, bf16 for S4) instead of casting
    on-chip; W_out/enc/maskbias/ones/bias are bf16 host-side.
"""

import os

import numpy as np

import concourse.bass as bass
import concourse.mybir as mybir
import concourse.tile as tile
from concourse import bacc
from concourse.bass_utils import run_bass_kernel_spmd
from concourse.masks import make_identity

# Problem shape (hardcoded per the harness contract).
B, T, S, H = 32, 128, 512, 1024
NCORES = 8
NB = B // NCORES          # batches (slots) per core
TB = NB * T               # stacked query rows per core (512)
K2 = 2 * H
NEG = np.float32(-1e9)

P = 128                   # SBUF/PSUM partitions
KH = H // P               # 8 k-tiles over H
NHALF = H // 512          # 2 PSUM-bank halves of H

F32 = mybir.dt.float32
F32R = mybir.dt.float32r
BF16 = mybir.dt.bfloat16

PAD = int(os.environ.get("KERNEL_PAD", "32"))
S1_BF16 = os.environ.get("KERNEL_S1_BF16", "1") == "1"
WARMUP_MMS = int(os.environ.get("KERNEL_WARMUP_MMS", "8"))
ENC_DMA_FROM = int(os.environ.get("KERNEL_ENC_DMA_FROM", "1"))
XBAR_PT = os.environ.get("KERNEL_XBAR_PT", "1") == "1"

S1_DT = BF16 if S1_BF16 else F32R
NP_S1 = np.dtype(mybir.dt.np(S1_DT))
NP_BF16 = np.dtype(mybir.dt.np(BF16))


def _slot_plan(lens):
    """Sort batches by length (desc), deal round-robin to cores.

    Returns (order, slot_lens): order[j*NCORES + c] is the original batch
    index placed on core c, slot j; slot_lens[j] is the padded source length
    traced for slot j (max over the cores sharing that slot).
    """
    lens = np.asarray(lens, dtype=np.int64)
    order = np.argsort(-lens, kind="stable")
    pad = np.clip(np.ceil(lens[order] / PAD).astype(np.int64) * PAD, PAD, S)
    slot_lens = tuple(
        int(pad[j * NCORES : (j + 1) * NCORES].max()) for j in range(NB)
    )
    return order, slot_lens


def _emit(nc, tc, slot_lens):
    X = mybir.AxisListType
    AF = mybir.ActivationFunctionType
    ts = bass.ts

    qT_d = nc.dram_tensor("qT", [H, TB], S1_DT, kind="ExternalInput").ap()
    qTb_d = nc.dram_tensor("qTb", [H, TB], BF16, kind="ExternalInput").ap()
    winT_d = nc.dram_tensor("winT", [H, H], S1_DT, kind="ExternalInput").ap()
    woutT_d = nc.dram_tensor("woutT", [K2, H], BF16, kind="ExternalInput").ap()
    encT_d = [
        nc.dram_tensor(f"encT{b}", [H, slot_lens[b]], BF16, kind="ExternalInput").ap()
        for b in range(NB)
    ]
    # natural-layout enc for slots >= ENC_DMA_FROM (S3 rhs comes straight
    # from DRAM instead of PE transposes; slot 0's window is DMA-tight so it
    # keeps the on-chip transpose path)
    encN_d = [
        nc.dram_tensor(f"encN{b}", [slot_lens[b], H], BF16, kind="ExternalInput").ap()
        if b >= ENC_DMA_FROM else None
        for b in range(NB)
    ]
    mb_d = nc.dram_tensor("maskbias", [NB, S], BF16, kind="ExternalInput").ap()
    bias_d = nc.dram_tensor("bias", [H], BF16, kind="ExternalInput").ap()
    ones_d = nc.dram_tensor("ones", [P], BF16, kind="ExternalInput").ap()
    out_d = nc.dram_tensor("out", [NB, T, H], F32, kind="ExternalOutput").ap()

    # per-slot geometry: number of 128-chunks and width of the last chunk
    KS = [(L + P - 1) // P for L in slot_lens]
    REM = [L - P * (k - 1) for L, k in zip(slot_lens, KS)]

    with (
        tc.tile_pool(name="sb", bufs=1) as sb,
        tc.tile_pool(name="ps", bufs=1, space="PSUM") as ps,
    ):
        # ---- persistent SBUF tiles ----
        qT_sb = sb.tile([P, KH, TB], S1_DT, tag="qT")
        qpT_sb = sb.tile([P, KH, TB], BF16, tag="qpT")
        qTb_sb = sb.tile([P, KH, TB], BF16, tag="qTb")
        w_out_sb = sb.tile([P, 2 * KH, H], BF16, tag="wout")
        ones_sb = sb.tile([1, P], BF16, tag="ones")
        bias_sb = sb.tile([1, H], BF16, tag="bias")
        mb_sb = sb.tile([1, NB, S], BF16, tag="mb")
        idr_sb = sb.tile([P, P], BF16, tag="idr")
        scratch = sb.tile([P, 512], BF16, tag="scratch")

        # PSUM: one pool, 8 banks via same-size tags ([P,512] f32 = 2KB).
        PS_TAGS = [f"b{i}" for i in range(8)]

        def ps_tile(i, name, dt=F32, shape=(P, 512)):
            return ps.tile(list(shape), dt, tag=PS_TAGS[i], name=name,
                           padded_shape=[P, 512] if dt == F32 else [P, 1024])

        # warmup scratch memset is quick; warmup matmuls release the PE HAM
        # clock gate while the first DMAs stream in.  Small input DMAs go on
        # the (otherwise idle) gpsimd queue to keep sync free for the big
        # streaming loads.
        nc.gpsimd.memset(scratch[:].bitcast(F32), 0.0)
        make_identity(nc, idr_sb[:])
        nc.scalar.dma_start(out=ones_sb[:], in_=ones_d[None, :])
        nc.scalar.dma_start(out=mb_sb[:], in_=mb_d[None, :, :])
        nc.scalar.dma_start(out=bias_sb[:], in_=bias_d[None, :])

        # ---- S1: q_projT = W_in^T-blocks @ qT, kh-outer over all 8 banks.
        qTb_r = qTb_d.rearrange("(kh p) t -> p kh t", p=P)
        woutT_r = woutT_d.rearrange("(g kk p) h -> g p kk h", p=P, g=4)

        qp_ps = [ps_tile(mg, f"qp_ps{mg}") for mg in range(KH)]
        if WARMUP_MMS:
            with nc.named_scope("warmup"):
                for _ in range(WARMUP_MMS):
                    nc.tensor.matmul(
                        qp_ps[7][:], scratch[:, 0:P], scratch[:],
                        start=True, stop=True, skip_group_check=True,
                    )

        def _load_encT(j):
            t = sb.tile([P, KH, slot_lens[j]], BF16, tag="encT", bufs=3,
                        name=f"encT_sb{j}")
            nc.sync.dma_start(
                out=t[:], in_=encT_d[j].rearrange("(kh p) s -> p kh s", p=P)
            )
            return t

        encT_sb = [None] * NB
        encN_sb = [None] * NB
        qT_rk = qT_d.rearrange("(kh p) t -> p kh t", p=P)
        winT_rp = winT_d.rearrange("(kh p) g -> p kh g", p=P)

        def _load_encN(j):
            """Natural-layout enc rows straight from DRAM (gpsimd queue)."""
            t = sb.tile([P, KS[j], H], BF16, tag="encN", bufs=3,
                        name=f"encN_sb{j}")
            encN_sb[j] = t
            nfull = KS[j] - 1
            if nfull:
                nc.scalar.dma_start(
                    out=t[:, 0:nfull, :],
                    in_=encN_d[j][0 : nfull * P, :].rearrange(
                        "(ks p) h -> p ks h", p=P
                    ),
                )
            nc.scalar.dma_start(
                out=t[0 : REM[j], nfull, :], in_=encN_d[j][nfull * P :, :]
            )

        with nc.named_scope("s1"):
            # S1 streaming loads: fine-grained first chunks so the first
            # matmul group starts ASAP, coarser after.  Everything else is
            # dispatched behind them in first-use order.
            WCH = [(0, 1), (1, 2), (2, 4), (4, 6), (6, 8)]
            w_t = {}
            nc.sync.dma_start(out=qT_sb[:, 0:1, :], in_=qT_rk[:, 0:1, :])
            for i, (lo, hi) in enumerate(WCH):
                t = sb.tile([P, hi - lo, H], S1_DT, tag="win", bufs=5,
                            name=f"w_t{i}")
                for kh in range(lo, hi):
                    w_t[kh] = (t, kh - lo)
                nc.sync.dma_start(out=t[:], in_=winT_rp[:, lo:hi, :])
                if i == 0:
                    nc.sync.dma_start(out=qT_sb[:, 1:4, :], in_=qT_rk[:, 1:4, :])
                elif i == 2:
                    nc.sync.dma_start(out=qT_sb[:, 4:8, :], in_=qT_rk[:, 4:8, :])
            encT_sb[0] = _load_encT(0)
            encT_sb[1] = _load_encT(1)
            nc.sync.dma_start(out=qTb_sb[:], in_=qTb_r[:])
            for g in range(4):
                nc.sync.dma_start(
                    out=w_out_sb[:, 4 * g : 4 * g + 4, :], in_=woutT_r[g]
                )
            for j in range(ENC_DMA_FROM, 2):
                _load_encN(j)
            for kh in range(KH):
                wt, wi = w_t[kh]
                for mg in range(KH):
                    nc.tensor.matmul(
                        qp_ps[mg][:],
                        wt[:, wi, ts(mg, P)],
                        qT_sb[:, kh, :],
                        start=(kh == 0),
                        stop=(kh == KH - 1),
                    )
                    if kh == KH - 1:
                        nc.vector.tensor_copy(qpT_sb[:, mg, :], qp_ps[mg][:])

        # ---- slot-loop state ----
        # tr-bank rotation for transpose staging (banks 6, 7)
        tr_state = [6]

        def tr_tile(name, dt=BF16, shape=(P, 4, P)):
            i = tr_state[0]
            tr_state[0] = 13 - i  # 6 <-> 7
            return ps.tile(list(shape), dt, tag=PS_TAGS[i], name=name,
                           padded_shape=[P, 4, P])

        p_sb = [None] * NB
        pT_sb = [None] * NB
        rinv = [None] * NB
        c_sb = [None] * NB
        cT_sb = [None] * NB
        o_ps = [None] * NB

        def emit_encNtr(j):
            """encN[j] (natural [s,h] tiles) from encT[j] via PE transposes."""
            t = sb.tile([P, KS[j], H], BF16, tag="encN", bufs=3,
                        name=f"encN_sb{j}")
            encN_sb[j] = t
            for ks in range(KS[j]):
                cw = REM[j] if ks == KS[j] - 1 else P
                for half in range(2):
                    e_ps = tr_tile(f"encNtr{j}_{ks}_{half}")
                    for i in range(4):
                        ih = half * 4 + i
                        nc.tensor.transpose(
                            e_ps[0:cw, i, :],
                            encT_sb[j][:, ih, ks * P : ks * P + cw],
                            idr_sb[:],
                        )
                    nc.vector.tensor_copy(
                        t[0:cw, ks, half * 512 : half * 512 + 512],
                        e_ps[0:cw, :, :],
                    )

        def emit_s2(j):
            """score(j) + additive length mask into sm bank (j%2)."""
            Ln = slot_lens[j]
            sc = ps_tile(j % 2, f"score{j}", shape=(P, Ln))
            for kh in range(KH):
                nc.tensor.matmul(
                    sc[:],
                    qpT_sb[:, kh, ts(j, T)],
                    encT_sb[j][:, kh, :],
                    start=(kh == 0),
                    stop=False,
                )
            nc.tensor.matmul(
                sc[:], ones_sb[:], mb_sb[:, j, 0:Ln], start=False, stop=True
            )
            return sc

        def emit_softmax(j, sc):
            Ln = slot_lens[j]
            negmax = sb.tile([P, 1], F32, tag="negmax", bufs=2, name="negmax")
            nc.vector.reduce_max(negmax[:], sc[:], axis=X.X, negate=True)
            p_sb[j] = sb.tile([P, KS[j] * P], BF16, tag="p", bufs=2,
                              name=f"p{j}")
            rowsum = sb.tile([P, 1], F32, tag="rowsum", bufs=2, name="rowsum")
            nc.scalar.activation(
                p_sb[j][:, 0:Ln], sc[:], AF.Exp, bias=negmax[:],
                accum_out=rowsum[:],
            )
            rinvb = sb.tile([P, 1], F32, tag="rinv", bufs=2, name=f"rinv{j}")
            nc.vector.reciprocal(rinvb[:], rowsum[:])
            # diag(1/rowsum): used as the cT-transpose "identity" so the
            # softmax normalization rides along for free.
            rinv[j] = sb.tile([P, P], BF16, tag="diag", bufs=2,
                              name=f"diag{j}")
            nc.vector.tensor_scalar_mul(rinv[j][:], idr_sb[:], rinvb[:])


        def emit_prefix(j):
            """S4 query-half: [q] @ W_out_q into a-banks (2, 3); no stop."""
            o_ps[j] = []
            for nh in range(NHALF):
                o = ps_tile(2 + nh, f"o_ps{j}_{nh}")
                o_ps[j].append(o)
                for kk in range(KH):
                    nc.tensor.matmul(
                        o[:],
                        qTb_sb[:, kk, ts(j, T)],
                        w_out_sb[:, kk, ts(nh, 512)],
                        start=(kk == 0),
                        stop=False,
                    )

        def emit_pT(j):
            """p -> pT chunks via PE transposes (one staging tile).  Reads
            full 128-chunks of the padded p tile; garbage columns land in
            partitions >= REM which S3 never reads."""
            pT_ps = tr_tile(f"pTtr{j}")
            for ks in range(KS[j]):
                nc.tensor.transpose(
                    pT_ps[:, ks, :], p_sb[j][:, ks * P : (ks + 1) * P],
                    idr_sb[:],
                )
            pT_sb[j] = sb.tile([P, KS[j], P], BF16, tag="pT", bufs=2,
                               name=f"pT{j}")
            nc.vector.tensor_copy(pT_sb[j][:], pT_ps[:, 0 : KS[j], :])

        def emit_s3(j):
            """c~ = pT^T @ encN into c banks (4, 5); plain-copy evict (the
            1/rowsum normalization is folded into the cT transpose)."""
            c_sb[j] = sb.tile([P, H], BF16, tag="c", bufs=2, name=f"c{j}")
            for nh in range(NHALF):
                c_ps = ps_tile(4 + nh, f"c_ps{j}_{nh}")
                for ks in range(KS[j]):
                    cw = REM[j] if ks == KS[j] - 1 else P
                    nc.tensor.matmul(
                        c_ps[:],
                        pT_sb[j][0:cw, ks, :],
                        encN_sb[j][0:cw, ks, ts(nh, 512)],
                        start=(ks == 0),
                        stop=(ks == KS[j] - 1),
                    )
                nc.vector.tensor_copy(c_sb[j][:, ts(nh, 512)], c_ps[:])

        def emit_cT(j):
            """c~ -> cT via regular matmuls against diag(1/rowsum): the PE
            transpose mode ignores its rhs, but c~^T @ diag(rinv) as a plain
            matmul transposes AND normalizes in one pass."""
            cT_sb[j] = sb.tile([P, KH, P], BF16, tag="cT", bufs=2,
                               name=f"cT{j}")
            for half in range(2):
                cT_ps = tr_tile(f"cTtr{j}_{half}", dt=F32)
                for i in range(4):
                    nc.tensor.matmul(
                        cT_ps[:, i, :], c_sb[j][:, ts(half * 4 + i, P)],
                        rinv[j][:], start=True, stop=True,
                    )
                nc.vector.tensor_copy(
                    cT_sb[j][:, half * 4 : half * 4 + 4, :], cT_ps[:]
                )

        def emit_suffix(j):
            """S4 context-half + bias; tanh-evict and store per nh half."""
            out_sb = sb.tile([P, H], F32, tag="out", bufs=2, name=f"out{j}")
            for nh in range(NHALF):
                nsl = ts(nh, 512)
                for kk in range(KH):
                    nc.tensor.matmul(
                        o_ps[j][nh][:],
                        cT_sb[j][:, kk, :],
                        w_out_sb[:, KH + kk, nsl],
                        start=False,
                        stop=False,
                    )
                nc.tensor.matmul(
                    o_ps[j][nh][:], ones_sb[:], bias_sb[:, nsl],
                    start=False, stop=True,
                )
                nc.scalar.activation(out_sb[:, nsl], o_ps[j][nh][:], AF.Tanh)
                nc.scalar.dma_start(out=out_d[j][:, nsl], in_=out_sb[:, nsl])

        # ---- prologue: slot 0 head; encNtr(0) + prefix(0) cover the
        # softmax(0) latency (S2(0) needs only encT0, so it goes first).
        with nc.named_scope("b0h"):
            sc0 = emit_s2(0)
            emit_softmax(0, sc0)
            if ENC_DMA_FROM > 0:
                emit_encNtr(0)
            else:
                _load_encN(0)
            emit_prefix(0)

        # ---- software-pipelined slot loop ----
        for b in range(NB):
            scope = nc.named_scope(f"b{b}")
            scope.__enter__()
            emit_pT(b)
            emit_s3(b)
            emit_cT(b)
            if b + 1 < NB:
                if b + 2 < NB:
                    encT_sb[b + 2] = _load_encT(b + 2)
                    if b + 2 >= ENC_DMA_FROM:
                        _load_encN(b + 2)
                if b + 1 < ENC_DMA_FROM:
                    emit_encNtr(b + 1)
                sc = emit_s2(b + 1)
                emit_softmax(b + 1, sc)
            emit_suffix(b)
            if b + 1 < NB:
                emit_prefix(b + 1)
            scope.__exit__(None, None, None)


def build_nc(slot_lens=(S,) * NB):
    # Bacc (not raw Bass): its lowering splits multi-sem waits and moves
    # matmul waits onto ldweights, which TRN2 codegen requires.
    nc = bacc.Bacc("TRN2", target_bir_lowering=False, debug=False)
    with tile.TileContext(nc) as tc:
        _emit(nc, tc, slot_lens)
    nc.compile()
    return nc


_NC_CACHE = {}


def _get_nc(slot_lens):
    key = (S1_DT, PAD, ENC_DMA_FROM, slot_lens)
    if key not in _NC_CACHE:
        _NC_CACHE[key] = build_nc(slot_lens)
    return _NC_CACHE[key]


def make_in_maps(query, encoder_outputs, src_lengths, W_in, W_out, b_out):
    """Host-side sharding + layout prep.

    Returns (in_maps, order, slot_lens): one input map per core; order maps
    (slot j, core c) -> original batch index order[j*NCORES + c].
    """
    query = np.asarray(query, dtype=np.float32)
    enc = np.asarray(encoder_outputs, dtype=np.float32)
    lens = np.asarray(src_lengths, dtype=np.int32)
    order, slot_lens = _slot_plan(lens)

    w_inT = np.ascontiguousarray(np.asarray(W_in, dtype=np.float32).T).astype(NP_S1)
    w_outT = np.ascontiguousarray(np.asarray(W_out, dtype=np.float32).T).astype(NP_BF16)
    bias = np.asarray(b_out, dtype=np.float32).astype(NP_BF16)
    ones = np.ones((P,), dtype=NP_BF16)

    in_maps = []
    for c in range(NCORES):
        idx = [int(order[j * NCORES + c]) for j in range(NB)]
        q_c = query[idx]                      # [NB, T, H] in slot order
        qT = np.ascontiguousarray(q_c.transpose(2, 0, 1)).reshape(H, TB)
        maskbias = np.where(
            np.arange(S, dtype=np.int64)[None, :]
            < lens[idx][:, None].astype(np.int64),
            np.float32(0.0),
            NEG,
        ).astype(NP_BF16)
        im = {
            "qT": qT.astype(NP_S1),
            "qTb": qT.astype(NP_BF16),
            "winT": w_inT,
            "woutT": w_outT,
            "maskbias": maskbias,
            "bias": bias,
            "ones": ones,
        }
        for j in range(NB):
            Ln = slot_lens[j]
            e_b = enc[idx[j], :Ln, :]         # [Ln, H]
            im[f"encT{j}"] = np.ascontiguousarray(e_b.T).astype(NP_BF16)
            if j >= ENC_DMA_FROM:
                im[f"encN{j}"] = np.ascontiguousarray(e_b).astype(NP_BF16)
        in_maps.append(im)
    return in_maps, order, slot_lens


def run(query, encoder_outputs, src_lengths, W_in, W_out, b_out, **spmd_kwargs):
    in_maps, order, slot_lens = make_in_maps(
        query, encoder_outputs, src_lengths, W_in, W_out, b_out
    )
    res = run_bass_kernel_spmd(
        _get_nc(slot_lens), in_maps, list(range(NCORES)), **spmd_kwargs
    )
    out = np.empty((B, T, H), dtype=np.float32)
    for c in range(NCORES):
        core_out = res.results[c]["out"]      # [NB, T, H] in slot order
        for j in range(NB):
            out[int(order[j * NCORES + c])] = core_out[j]
    return out, res


def kernel(query, encoder_outputs, src_lengths, W_in, W_out, b_out):
    out, _ = run(query, encoder_outputs, src_lengths, W_in, W_out, b_out)
    return out


# revision 46
# speedup vs baseline: 1.2162x; 1.0051x over previous
"""Masked cross-attention + linear_in/linear_out, fused Trainium2 kernel (v2).

Problem (nn_Attention_50096498541174):
    q_proj = query @ W_in.T                         [B,T,H]
    score  = q_proj @ enc.T  (masked by src_lengths)[B,T,S]
    p      = softmax(score, -1)
    c      = p @ enc                                [B,T,H]
    out    = tanh(concat(query, c) @ W_out.T + b)   [B,T,H]

Sharding: data-parallel over batch B=32 across 8 NeuronCores (4 slots/core),
weights replicated, no collectives.  Batches are sorted by src_length (desc)
and dealt round-robin to cores; each slot is traced for the max padded length
over the cores sharing it (same program per core -> one SPMD NEFF).

v2 layout/dtype plan (vs the f32r baseline):
  - S1 (q_proj) stays f32r for accuracy: score errors are the dominant
    error term and qp feeds the scores.  Everything downstream is bf16
    (enc, qp, p, c, W_out), halving the big DMA transfers and running all
    PE transposes at 1 cycle/row.  Expected rel err ~1e-2 (gate 2e-2).
  - Lengths padded to 32 (not 128): S2/S3/enc-transposes shrink ~30% on
    average; partial 128-blocks handled with partial-partition matmuls.
  - One PSUM pool, 8 banks hand-assigned via tags; one SBUF data pool.
    (Each tile pool costs a ~0.7us teardown barrier round at kernel end.)
  - Software-pipelined slot loop; per-engine queue order chosen so the
    softmax latency chain (DVE max -> ACT exp -> DVE recip) and all PSUM
    evictions hide under independent PE work from the adjacent slots:
      PE:  pT(b) S3(b) cT(b) encNtr(b+1) S2(b+1) S4suf(b) S4pre(b+1)
      DVE: pT-ev c-ev cT-ev encN-ev rmax recip   (emission order)
      ACT: exp(b+1) tanh(b)
  - query is DMA'd twice (f32r for S1, bf16 for S4) instead of casting
    on-chip; W_out/enc/maskbias/ones/bias are bf16 host-side.
"""

import os

import numpy as np

import concourse.bass as bass
import concourse.mybir as mybir
import concourse.tile as tile
from concourse import bacc
from concourse.bass_utils import run_bass_kernel_spmd
from concourse.masks import make_identity

# Problem shape (hardcoded per the harness contract).
B, T, S, H = 32, 128, 512, 1024
NCORES = 8
NB = B // NCORES          # batches (slots) per core
TB = NB * T               # stacked query rows per core (512)
K2 = 2 * H
NEG = np.float32(-1e9)

P = 128                   # SBUF/PSUM partitions
KH = H // P               # 8 k-tiles over H
NHALF = H // 512          # 2 PSUM-bank halves of H

F32 = mybir.dt.float32
F32R = mybir.dt.float32r
BF16 = mybir.dt.bfloat16

PAD = int(os.environ.get("KERNEL_PAD", "32"))
S1_BF16 = os.environ.get("KERNEL_S1_BF16", "1") == "1"
WARMUP_MMS = int(os.environ.get("KERNEL_WARMUP_MMS", "8"))
ENC_DMA_FROM = int(os.environ.get("KERNEL_ENC_DMA_FROM", "1"))
XBAR_PT = os.environ.get("KERNEL_XBAR_PT", "1") == "1"

S1_DT = BF16 if S1_BF16 else F32R
NP_S1 = np.dtype(mybir.dt.np(S1_DT))
NP_BF16 = np.dtype(mybir.dt.np(BF16))


def _slot_plan(lens):
    """Sort batches by length (desc), deal round-robin to cores.

    Returns (order, slot_lens): order[j*NCORES + c] is the original batch
    index placed on core c, slot j; slot_lens[j] is the padded source length
    traced for slot j (max over the cores sharing that slot).
    """
    lens = np.asarray(lens, dtype=np.int64)
    order = np.argsort(-lens, kind="stable")
    pad = np.clip(np.ceil(lens[order] / PAD).astype(np.int64) * PAD, PAD, S)
    slot_lens = tuple(
        int(pad[j * NCORES : (j + 1) * NCORES].max()) for j in range(NB)
    )
    return order, slot_lens


def _emit(nc, tc, slot_lens):
    X = mybir.AxisListType
    AF = mybir.ActivationFunctionType
    ts = bass.ts

    qT_d = nc.dram_tensor("qT", [H, TB], S1_DT, kind="ExternalInput").ap()
    qTb_d = (None if S1_BF16 else
             nc.dram_tensor("qTb", [H, TB], BF16, kind="ExternalInput").ap())
    winT_d = nc.dram_tensor("winT", [H, H], S1_DT, kind="ExternalInput").ap()
    woutT_d = nc.dram_tensor("woutT", [K2, H], BF16, kind="ExternalInput").ap()
    encT_d = [
        nc.dram_tensor(f"encT{b}", [H, slot_lens[b]], BF16, kind="ExternalInput").ap()
        for b in range(NB)
    ]
    # natural-layout enc for slots >= ENC_DMA_FROM (S3 rhs comes straight
    # from DRAM instead of PE transposes; slot 0's window is DMA-tight so it
    # keeps the on-chip transpose path)
    encN_d = [
        nc.dram_tensor(f"encN{b}", [slot_lens[b], H], BF16, kind="ExternalInput").ap()
        if b >= ENC_DMA_FROM else None
        for b in range(NB)
    ]
    mb_d = nc.dram_tensor("maskbias", [NB, S], BF16, kind="ExternalInput").ap()
    bias_d = nc.dram_tensor("bias", [H], BF16, kind="ExternalInput").ap()
    ones_d = nc.dram_tensor("ones", [P], BF16, kind="ExternalInput").ap()
    out_d = nc.dram_tensor("out", [NB, T, H], F32, kind="ExternalOutput").ap()

    # per-slot geometry: number of 128-chunks and width of the last chunk
    KS = [(L + P - 1) // P for L in slot_lens]
    REM = [L - P * (k - 1) for L, k in zip(slot_lens, KS)]

    with (
        tc.tile_pool(name="sb", bufs=1) as sb,
        tc.tile_pool(name="ps", bufs=1, space="PSUM") as ps,
    ):
        # ---- persistent SBUF tiles ----
        qT_sb = sb.tile([P, KH, TB], S1_DT, tag="qT")
        qpT_sb = sb.tile([P, KH, TB], BF16, tag="qpT")
        # with bf16 S1, the S4 prefix reads qT_sb directly (same layout/dtype)
        qTb_sb = qT_sb if S1_BF16 else sb.tile([P, KH, TB], BF16, tag="qTb")
        w_out_sb = sb.tile([P, 2 * KH, H], BF16, tag="wout")
        ones_sb = sb.tile([1, P], BF16, tag="ones")
        bias_sb = sb.tile([1, H], BF16, tag="bias")
        mb_sb = sb.tile([1, NB, S], BF16, tag="mb")
        idr_sb = sb.tile([P, P], BF16, tag="idr")
        scratch = sb.tile([P, 512], BF16, tag="scratch")

        # PSUM: one pool, 8 banks via same-size tags ([P,512] f32 = 2KB).
        PS_TAGS = [f"b{i}" for i in range(8)]

        def ps_tile(i, name, dt=F32, shape=(P, 512)):
            return ps.tile(list(shape), dt, tag=PS_TAGS[i], name=name,
                           padded_shape=[P, 512] if dt == F32 else [P, 1024])

        # warmup scratch memset is quick; warmup matmuls release the PE HAM
        # clock gate while the first DMAs stream in.  Small input DMAs go on
        # the (otherwise idle) gpsimd queue to keep sync free for the big
        # streaming loads.
        nc.gpsimd.memset(scratch[:].bitcast(F32), 0.0)
        make_identity(nc, idr_sb[:])
        nc.scalar.dma_start(out=ones_sb[:], in_=ones_d[None, :])
        nc.scalar.dma_start(out=mb_sb[:], in_=mb_d[None, :, :])
        nc.scalar.dma_start(out=bias_sb[:], in_=bias_d[None, :])

        # ---- S1: q_projT = W_in^T-blocks @ qT, kh-outer over all 8 banks.
        qTb_r = None if S1_BF16 else qTb_d.rearrange("(kh p) t -> p kh t", p=P)
        woutT_r = woutT_d.rearrange("(g kk p) h -> g p kk h", p=P, g=4)

        qp_ps = [ps_tile(mg, f"qp_ps{mg}") for mg in range(KH)]
        if WARMUP_MMS:
            with nc.named_scope("warmup"):
                for _ in range(WARMUP_MMS):
                    nc.tensor.matmul(
                        qp_ps[7][:], scratch[:, 0:P], scratch[:],
                        start=True, stop=True, skip_group_check=True,
                    )

        def _load_encT(j):
            t = sb.tile([P, KH, slot_lens[j]], BF16, tag="encT", bufs=3,
                        name=f"encT_sb{j}")
            nc.sync.dma_start(
                out=t[:], in_=encT_d[j].rearrange("(kh p) s -> p kh s", p=P)
            )
            return t

        encT_sb = [None] * NB
        encN_sb = [None] * NB
        qT_rk = qT_d.rearrange("(kh p) t -> p kh t", p=P)
        winT_rp = winT_d.rearrange("(kh p) g -> p kh g", p=P)

        def _load_encN(j):
            """Natural-layout enc rows straight from DRAM (gpsimd queue)."""
            t = sb.tile([P, KS[j], H], BF16, tag="encN", bufs=3,
                        name=f"encN_sb{j}")
            encN_sb[j] = t
            nfull = KS[j] - 1
            if nfull:
                nc.scalar.dma_start(
                    out=t[:, 0:nfull, :],
                    in_=encN_d[j][0 : nfull * P, :].rearrange(
                        "(ks p) h -> p ks h", p=P
                    ),
                )
            nc.scalar.dma_start(
                out=t[0 : REM[j], nfull, :], in_=encN_d[j][nfull * P :, :]
            )

        with nc.named_scope("s1"):
            # S1 streaming loads: fine-grained first chunks so the first
            # matmul group starts ASAP, coarser after.  Everything else is
            # dispatched behind them in first-use order.
            WCH = [(0, 1), (1, 2), (2, 4), (4, 6), (6, 8)]
            w_t = {}
            nc.sync.dma_start(out=qT_sb[:, 0:1, :], in_=qT_rk[:, 0:1, :])
            for i, (lo, hi) in enumerate(WCH):
                t = sb.tile([P, hi - lo, H], S1_DT, tag="win", bufs=5,
                            name=f"w_t{i}")
                for kh in range(lo, hi):
                    w_t[kh] = (t, kh - lo)
                nc.sync.dma_start(out=t[:], in_=winT_rp[:, lo:hi, :])
                if i == 0:
                    nc.sync.dma_start(out=qT_sb[:, 1:4, :], in_=qT_rk[:, 1:4, :])
                elif i == 2:
                    nc.sync.dma_start(out=qT_sb[:, 4:8, :], in_=qT_rk[:, 4:8, :])
            encT_sb[0] = _load_encT(0)
            encT_sb[1] = _load_encT(1)
            if not S1_BF16:
                nc.sync.dma_start(out=qTb_sb[:], in_=qTb_r[:])
            for g in range(4):
                nc.sync.dma_start(
                    out=w_out_sb[:, 4 * g : 4 * g + 4, :], in_=woutT_r[g]
                )
            for j in range(ENC_DMA_FROM, 2):
                _load_encN(j)
            for kh in range(KH):
                wt, wi = w_t[kh]
                for mg in range(KH):
                    nc.tensor.matmul(
                        qp_ps[mg][:],
                        wt[:, wi, ts(mg, P)],
                        qT_sb[:, kh, :],
                        start=(kh == 0),
                        stop=(kh == KH - 1),
                    )
                    if kh == KH - 1:
                        nc.vector.tensor_copy(qpT_sb[:, mg, :], qp_ps[mg][:])

        # ---- slot-loop state ----
        # tr-bank rotation for transpose staging (banks 6, 7)
        tr_state = [6]

        def tr_tile(name, dt=BF16, shape=(P, 4, P)):
            i = tr_state[0]
            tr_state[0] = 13 - i  # 6 <-> 7
            return ps.tile(list(shape), dt, tag=PS_TAGS[i], name=name,
                           padded_shape=[P, 4, P])

        p_sb = [None] * NB
        pT_sb = [None] * NB
        rinv = [None] * NB
        c_sb = [None] * NB
        cT_sb = [None] * NB
        o_ps = [None] * NB

        def emit_encNtr(j):
            """encN[j] (natural [s,h] tiles) from encT[j] via PE transposes."""
            t = sb.tile([P, KS[j], H], BF16, tag="encN", bufs=3,
                        name=f"encN_sb{j}")
            encN_sb[j] = t
            for ks in range(KS[j]):
                cw = REM[j] if ks == KS[j] - 1 else P
                for half in range(2):
                    e_ps = tr_tile(f"encNtr{j}_{ks}_{half}")
                    for i in range(4):
                        ih = half * 4 + i
                        nc.tensor.transpose(
                            e_ps[0:cw, i, :],
                            encT_sb[j][:, ih, ks * P : ks * P + cw],
                            idr_sb[:],
                        )
                    nc.vector.tensor_copy(
                        t[0:cw, ks, half * 512 : half * 512 + 512],
                        e_ps[0:cw, :, :],
                    )

        def emit_s2(j):
            """score(j) + additive length mask into sm bank (j%2)."""
            Ln = slot_lens[j]
            sc = ps_tile(j % 2, f"score{j}", shape=(P, Ln))
            for kh in range(KH):
                nc.tensor.matmul(
                    sc[:],
                    qpT_sb[:, kh, ts(j, T)],
                    encT_sb[j][:, kh, :],
                    start=(kh == 0),
                    stop=False,
                )
            nc.tensor.matmul(
                sc[:], ones_sb[:], mb_sb[:, j, 0:Ln], start=False, stop=True
            )
            return sc

        def emit_softmax(j, sc):
            Ln = slot_lens[j]
            negmax = sb.tile([P, 1], F32, tag="negmax", bufs=2, name="negmax")
            nc.vector.reduce_max(negmax[:], sc[:], axis=X.X, negate=True)
            p_sb[j] = sb.tile([P, KS[j] * P], BF16, tag="p", bufs=2,
                              name=f"p{j}")
            rowsum = sb.tile([P, 1], F32, tag="rowsum", bufs=2, name="rowsum")
            nc.scalar.activation(
                p_sb[j][:, 0:Ln], sc[:], AF.Exp, bias=negmax[:],
                accum_out=rowsum[:],
            )
            rinvb = sb.tile([P, 1], F32, tag="rinv", bufs=2, name=f"rinv{j}")
            nc.vector.reciprocal(rinvb[:], rowsum[:])
            # diag(1/rowsum): used as the cT-transpose "identity" so the
            # softmax normalization rides along for free.
            rinv[j] = sb.tile([P, P], BF16, tag="diag", bufs=2,
                              name=f"diag{j}")
            nc.vector.tensor_scalar_mul(rinv[j][:], idr_sb[:], rinvb[:])


        def emit_prefix(j):
            """S4 query-half: [q] @ W_out_q into a-banks (2, 3); no stop."""
            o_ps[j] = []
            for nh in range(NHALF):
                o = ps_tile(2 + nh, f"o_ps{j}_{nh}")
                o_ps[j].append(o)
                for kk in range(KH):
                    nc.tensor.matmul(
                        o[:],
                        qTb_sb[:, kk, ts(j, T)],
                        w_out_sb[:, kk, ts(nh, 512)],
                        start=(kk == 0),
                        stop=False,
                    )

        def emit_pT(j):
            """p -> pT chunks via PE transposes (one staging tile).  Reads
            full 128-chunks of the padded p tile; garbage columns land in
            partitions >= REM which S3 never reads."""
            pT_ps = tr_tile(f"pTtr{j}")
            for ks in range(KS[j]):
                nc.tensor.transpose(
                    pT_ps[:, ks, :], p_sb[j][:, ks * P : (ks + 1) * P],
                    idr_sb[:],
                )
            pT_sb[j] = sb.tile([P, KS[j], P], BF16, tag="pT", bufs=2,
                               name=f"pT{j}")
            nc.vector.tensor_copy(pT_sb[j][:], pT_ps[:, 0 : KS[j], :])

        def emit_s3(j):
            """c~ = pT^T @ encN into c banks (4, 5); plain-copy evict (the
            1/rowsum normalization is folded into the cT transpose)."""
            c_sb[j] = sb.tile([P, H], BF16, tag="c", bufs=2, name=f"c{j}")
            for nh in range(NHALF):
                c_ps = ps_tile(4 + nh, f"c_ps{j}_{nh}")
                for ks in range(KS[j]):
                    cw = REM[j] if ks == KS[j] - 1 else P
                    nc.tensor.matmul(
                        c_ps[:],
                        pT_sb[j][0:cw, ks, :],
                        encN_sb[j][0:cw, ks, ts(nh, 512)],
                        start=(ks == 0),
                        stop=(ks == KS[j] - 1),
                    )
                nc.vector.tensor_copy(c_sb[j][:, ts(nh, 512)], c_ps[:])

        def emit_cT(j):
            """c~ -> cT via regular matmuls against diag(1/rowsum): the PE
            transpose mode ignores its rhs, but c~^T @ diag(rinv) as a plain
            matmul transposes AND normalizes in one pass."""
            cT_sb[j] = sb.tile([P, KH, P], BF16, tag="cT", bufs=2,
                               name=f"cT{j}")
            for half in range(2):
                cT_ps = tr_tile(f"cTtr{j}_{half}", dt=F32)
                for i in range(4):
                    nc.tensor.matmul(
                        cT_ps[:, i, :], c_sb[j][:, ts(half * 4 + i, P)],
                        rinv[j][:], start=True, stop=True,
                    )
                nc.vector.tensor_copy(
                    cT_sb[j][:, half * 4 : half * 4 + 4, :], cT_ps[:]
                )

        def emit_suffix(j):
            """S4 context-half + bias; tanh-evict and store per nh half."""
            out_sb = sb.tile([P, H], F32, tag="out", bufs=2, name=f"out{j}")
            for nh in range(NHALF):
                nsl = ts(nh, 512)
                for kk in range(KH):
                    nc.tensor.matmul(
                        o_ps[j][nh][:],
                        cT_sb[j][:, kk, :],
                        w_out_sb[:, KH + kk, nsl],
                        start=False,
                        stop=False,
                    )
                nc.tensor.matmul(
                    o_ps[j][nh][:], ones_sb[:], bias_sb[:, nsl],
                    start=False, stop=True,
                )
                nc.scalar.activation(out_sb[:, nsl], o_ps[j][nh][:], AF.Tanh)
                nc.scalar.dma_start(out=out_d[j][:, nsl], in_=out_sb[:, nsl])

        # ---- prologue: slot 0 head; encNtr(0) + prefix(0) cover the
        # softmax(0) latency (S2(0) needs only encT0, so it goes first).
        with nc.named_scope("b0h"):
            sc0 = emit_s2(0)
            emit_softmax(0, sc0)
            if ENC_DMA_FROM > 0:
                emit_encNtr(0)
            else:
                _load_encN(0)
            emit_prefix(0)

        # ---- software-pipelined slot loop ----
        for b in range(NB):
            scope = nc.named_scope(f"b{b}")
            scope.__enter__()
            emit_pT(b)
            emit_s3(b)
            emit_cT(b)
            if b + 1 < NB:
                if b + 2 < NB:
                    encT_sb[b + 2] = _load_encT(b + 2)
                    if b + 2 >= ENC_DMA_FROM:
                        _load_encN(b + 2)
                if b + 1 < ENC_DMA_FROM:
                    emit_encNtr(b + 1)
                sc = emit_s2(b + 1)
                emit_softmax(b + 1, sc)
            emit_suffix(b)
            if b + 1 < NB:
                emit_prefix(b + 1)
            scope.__exit__(None, None, None)


def build_nc(slot_lens=(S,) * NB):
    # Bacc (not raw Bass): its lowering splits multi-sem waits and moves
    # matmul waits onto ldweights, which TRN2 codegen requires.
    nc = bacc.Bacc("TRN2", target_bir_lowering=False, debug=False)
    with tile.TileContext(nc) as tc:
        _emit(nc, tc, slot_lens)
    nc.compile()
    return nc


_NC_CACHE = {}


def _get_nc(slot_lens):
    key = (S1_DT, PAD, ENC_DMA_FROM, slot_lens)
    if key not in _NC_CACHE:
        _NC_CACHE[key] = build_nc(slot_lens)
    return _NC_CACHE[key]


def make_in_maps(query, encoder_outputs, src_lengths, W_in, W_out, b_out):
    """Host-side sharding + layout prep.

    Returns (in_maps, order, slot_lens): one input map per core; order maps
    (slot j, core c) -> original batch index order[j*NCORES + c].
    """
    query = np.asarray(query, dtype=np.float32)
    enc = np.asarray(encoder_outputs, dtype=np.float32)
    lens = np.asarray(src_lengths, dtype=np.int32)
    order, slot_lens = _slot_plan(lens)

    w_inT = np.ascontiguousarray(np.asarray(W_in, dtype=np.float32).T).astype(NP_S1)
    w_outT = np.ascontiguousarray(np.asarray(W_out, dtype=np.float32).T).astype(NP_BF16)
    bias = np.asarray(b_out, dtype=np.float32).astype(NP_BF16)
    ones = np.ones((P,), dtype=NP_BF16)

    in_maps = []
    for c in range(NCORES):
        idx = [int(order[j * NCORES + c]) for j in range(NB)]
        q_c = query[idx]                      # [NB, T, H] in slot order
        qT = np.ascontiguousarray(q_c.transpose(2, 0, 1)).reshape(H, TB)
        maskbias = np.where(
            np.arange(S, dtype=np.int64)[None, :]
            < lens[idx][:, None].astype(np.int64),
            np.float32(0.0),
            NEG,
        ).astype(NP_BF16)
        im = {
            "qT": qT.astype(NP_S1),
            **({} if S1_BF16 else {"qTb": qT.astype(NP_BF16)}),
            "winT": w_inT,
            "woutT": w_outT,
            "maskbias": maskbias,
            "bias": bias,
            "ones": ones,
        }
        for j in range(NB):
            Ln = slot_lens[j]
            e_b = enc[idx[j], :Ln, :]         # [Ln, H]
            im[f"encT{j}"] = np.ascontiguousarray(e_b.T).astype(NP_BF16)
            if j >= ENC_DMA_FROM:
                im[f"encN{j}"] = np.ascontiguousarray(e_b).astype(NP_BF16)
        in_maps.append(im)
    return in_maps, order, slot_lens


def run(query, encoder_outputs, src_lengths, W_in, W_out, b_out, **spmd_kwargs):
    in_maps, order, slot_lens = make_in_maps(
        query, encoder_outputs, src_lengths, W_in, W_out, b_out
    )
    res = run_bass_kernel_spmd(
        _get_nc(slot_lens), in_maps, list(range(NCORES)), **spmd_kwargs
    )
    out = np.empty((B, T, H), dtype=np.float32)
    for c in range(NCORES):
        core_out = res.results[c]["out"]      # [NB, T, H] in slot order
        for j in range(NB):
            out[int(order[j * NCORES + c])] = core_out[j]
    return out, res


def kernel(query, encoder_outputs, src_lengths, W_in, W_out, b_out):
    out, _ = run(query, encoder_outputs, src_lengths, W_in, W_out, b_out)
    return out


# revision 51
# speedup vs baseline: 1.2258x; 1.0078x over previous
"""Masked cross-attention + linear_in/linear_out, fused Trainium2 kernel (v2).

Problem (nn_Attention_50096498541174):
    q_proj = query @ W_in.T                         [B,T,H]
    score  = q_proj @ enc.T  (masked by src_lengths)[B,T,S]
    p      = softmax(score, -1)
    c      = p @ enc                                [B,T,H]
    out    = tanh(concat(query, c) @ W_out.T + b)   [B,T,H]

Sharding: data-parallel over batch B=32 across 8 NeuronCores (4 slots/core),
weights replicated, no collectives.  Batches are sorted by src_length (desc)
and dealt round-robin to cores; each slot is traced for the max padded length
over the cores sharing it (same program per core -> one SPMD NEFF).

v2 layout/dtype plan (vs the f32r baseline):
  - S1 (q_proj) stays f32r for accuracy: score errors are the dominant
    error term and qp feeds the scores.  Everything downstream is bf16
    (enc, qp, p, c, W_out), halving the big DMA transfers and running all
    PE transposes at 1 cycle/row.  Expected rel err ~1e-2 (gate 2e-2).
  - Lengths padded to 32 (not 128): S2/S3/enc-transposes shrink ~30% on
    average; partial 128-blocks handled with partial-partition matmuls.
  - One PSUM pool, 8 banks hand-assigned via tags; one SBUF data pool.
    (Each tile pool costs a ~0.7us teardown barrier round at kernel end.)
  - Software-pipelined slot loop; per-engine queue order chosen so the
    softmax latency chain (DVE max -> ACT exp -> DVE recip) and all PSUM
    evictions hide under independent PE work from the adjacent slots:
      PE:  pT(b) S3(b) cT(b) encNtr(b+1) S2(b+1) S4suf(b) S4pre(b+1)
      DVE: pT-ev c-ev cT-ev encN-ev rmax recip   (emission order)
      ACT: exp(b+1) tanh(b)
  - query is DMA'd twice (f32r for S1, bf16 for S4) instead of casting
    on-chip; W_out/enc/maskbias/ones/bias are bf16 host-side.
"""

import os

import numpy as np

import concourse.bass as bass
import concourse.mybir as mybir
import concourse.tile as tile
from concourse import bacc
from concourse.bass_utils import run_bass_kernel_spmd
from concourse.masks import make_identity

# Problem shape (hardcoded per the harness contract).
B, T, S, H = 32, 128, 512, 1024
NCORES = 8
NB = B // NCORES          # batches (slots) per core
TB = NB * T               # stacked query rows per core (512)
K2 = 2 * H
NEG = np.float32(-1e9)

P = 128                   # SBUF/PSUM partitions
KH = H // P               # 8 k-tiles over H
NHALF = H // 512          # 2 PSUM-bank halves of H

F32 = mybir.dt.float32
F32R = mybir.dt.float32r
BF16 = mybir.dt.bfloat16

PAD = int(os.environ.get("KERNEL_PAD", "32"))
S1_BF16 = os.environ.get("KERNEL_S1_BF16", "1") == "1"
WARMUP_MMS = int(os.environ.get("KERNEL_WARMUP_MMS", "6"))
ENC_DMA_FROM = int(os.environ.get("KERNEL_ENC_DMA_FROM", "1"))
XBAR_PT = os.environ.get("KERNEL_XBAR_PT", "1") == "1"

S1_DT = BF16 if S1_BF16 else F32R
NP_S1 = np.dtype(mybir.dt.np(S1_DT))
NP_BF16 = np.dtype(mybir.dt.np(BF16))


def _slot_plan(lens):
    """Sort batches by length (desc), deal round-robin to cores.

    Returns (order, slot_lens): order[j*NCORES + c] is the original batch
    index placed on core c, slot j; slot_lens[j] is the padded source length
    traced for slot j (max over the cores sharing that slot).
    """
    lens = np.asarray(lens, dtype=np.int64)
    order = np.argsort(-lens, kind="stable")
    pad = np.clip(np.ceil(lens[order] / PAD).astype(np.int64) * PAD, PAD, S)
    slot_lens = tuple(
        int(pad[j * NCORES : (j + 1) * NCORES].max()) for j in range(NB)
    )
    return order, slot_lens


def _emit(nc, tc, slot_lens):
    X = mybir.AxisListType
    AF = mybir.ActivationFunctionType
    ts = bass.ts

    qT_d = nc.dram_tensor("qT", [H, TB], S1_DT, kind="ExternalInput").ap()
    qTb_d = (None if S1_BF16 else
             nc.dram_tensor("qTb", [H, TB], BF16, kind="ExternalInput").ap())
    winT_d = nc.dram_tensor("winT", [H, H], S1_DT, kind="ExternalInput").ap()
    woutT_d = nc.dram_tensor("woutT", [K2, H], BF16, kind="ExternalInput").ap()
    encT_d = [
        nc.dram_tensor(f"encT{b}", [H, slot_lens[b]], BF16, kind="ExternalInput").ap()
        for b in range(NB)
    ]
    # natural-layout enc for slots >= ENC_DMA_FROM (S3 rhs comes straight
    # from DRAM instead of PE transposes; slot 0's window is DMA-tight so it
    # keeps the on-chip transpose path)
    encN_d = [
        nc.dram_tensor(f"encN{b}", [slot_lens[b], H], BF16, kind="ExternalInput").ap()
        if b >= ENC_DMA_FROM else None
        for b in range(NB)
    ]
    mb_d = nc.dram_tensor("maskbias", [NB, S], BF16, kind="ExternalInput").ap()
    bias_d = nc.dram_tensor("bias", [H], BF16, kind="ExternalInput").ap()
    ones_d = nc.dram_tensor("ones", [P], BF16, kind="ExternalInput").ap()
    out_d = nc.dram_tensor("out", [NB, T, H], F32, kind="ExternalOutput").ap()

    # per-slot geometry: number of 128-chunks and width of the last chunk
    KS = [(L + P - 1) // P for L in slot_lens]
    REM = [L - P * (k - 1) for L, k in zip(slot_lens, KS)]

    with (
        tc.tile_pool(name="sb", bufs=1) as sb,
        tc.tile_pool(name="ps", bufs=1, space="PSUM") as ps,
    ):
        # ---- persistent SBUF tiles ----
        qT_sb = sb.tile([P, KH, TB], S1_DT, tag="qT")
        qpT_sb = sb.tile([P, KH, TB], BF16, tag="qpT")
        # with bf16 S1, the S4 prefix reads qT_sb directly (same layout/dtype)
        qTb_sb = qT_sb if S1_BF16 else sb.tile([P, KH, TB], BF16, tag="qTb")
        w_out_sb = sb.tile([P, 2 * KH, H], BF16, tag="wout")
        ones_sb = sb.tile([1, P], BF16, tag="ones")
        bias_sb = sb.tile([1, H], BF16, tag="bias")
        mb_sb = sb.tile([1, NB, S], BF16, tag="mb")
        idr_sb = sb.tile([P, P], BF16, tag="idr")
        scratch = sb.tile([P, 512], BF16, tag="scratch")

        # PSUM: one pool, 8 banks via same-size tags ([P,512] f32 = 2KB).
        PS_TAGS = [f"b{i}" for i in range(8)]

        def ps_tile(i, name, dt=F32, shape=(P, 512)):
            return ps.tile(list(shape), dt, tag=PS_TAGS[i], name=name,
                           padded_shape=[P, 512] if dt == F32 else [P, 1024])

        # warmup scratch memset is quick; warmup matmuls release the PE HAM
        # clock gate while the first DMAs stream in.  Small input DMAs go on
        # the (otherwise idle) gpsimd queue to keep sync free for the big
        # streaming loads.
        nc.gpsimd.memset(scratch[:].bitcast(F32), 0.0)
        make_identity(nc, idr_sb[:])

        # ---- S1: q_projT = W_in^T-blocks @ qT, kh-outer over all 8 banks.
        qTb_r = None if S1_BF16 else qTb_d.rearrange("(kh p) t -> p kh t", p=P)
        woutT_r = woutT_d.rearrange("(g kk p) h -> g p kk h", p=P, g=4)

        qp_ps = [ps_tile(mg, f"qp_ps{mg}") for mg in range(KH)]
        if WARMUP_MMS:
            with nc.named_scope("warmup"):
                for _ in range(WARMUP_MMS):
                    nc.tensor.matmul(
                        qp_ps[7][:], scratch[:, 0:P], scratch[:],
                        start=True, stop=True, skip_group_check=True,
                    )

        def _load_encT(j):
            t = sb.tile([P, KH, slot_lens[j]], BF16, tag="encT", bufs=3,
                        name=f"encT_sb{j}")
            nc.sync.dma_start(
                out=t[:], in_=encT_d[j].rearrange("(kh p) s -> p kh s", p=P)
            )
            return t

        encT_sb = [None] * NB
        encN_sb = [None] * NB
        qT_rk = qT_d.rearrange("(kh p) t -> p kh t", p=P)
        winT_rp = winT_d.rearrange("(kh p) g -> p kh g", p=P)

        def _load_encN(j):
            """Natural-layout enc rows straight from DRAM (gpsimd queue)."""
            t = sb.tile([P, KS[j], H], BF16, tag="encN", bufs=3,
                        name=f"encN_sb{j}")
            encN_sb[j] = t
            nfull = KS[j] - 1
            if nfull:
                nc.scalar.dma_start(
                    out=t[:, 0:nfull, :],
                    in_=encN_d[j][0 : nfull * P, :].rearrange(
                        "(ks p) h -> p ks h", p=P
                    ),
                )
            nc.scalar.dma_start(
                out=t[0 : REM[j], nfull, :], in_=encN_d[j][nfull * P :, :]
            )

        with nc.named_scope("s1"):
            # S1 streaming loads: fine-grained first chunks so the first
            # matmul group starts ASAP, coarser after.  Everything else is
            # dispatched behind them in first-use order.
            # qT chunks ride the ACT hwdge queue (idle until softmax), winT
            # chunks the sync queue: the two streams dispatch in parallel.
            WCH = [(0, 1), (1, 2), (2, 4), (4, 6), (6, 8)]
            w_t = {}
            nc.scalar.dma_start(out=qT_sb[:, 0:1, :], in_=qT_rk[:, 0:1, :])
            nc.scalar.dma_start(out=qT_sb[:, 1:4, :], in_=qT_rk[:, 1:4, :])
            nc.scalar.dma_start(out=qT_sb[:, 4:8, :], in_=qT_rk[:, 4:8, :])
            nc.scalar.dma_start(out=ones_sb[:], in_=ones_d[None, :])
            nc.scalar.dma_start(out=mb_sb[:], in_=mb_d[None, :, :])
            nc.scalar.dma_start(out=bias_sb[:], in_=bias_d[None, :])
            for i, (lo, hi) in enumerate(WCH):
                t = sb.tile([P, hi - lo, H], S1_DT, tag="win", bufs=5,
                            name=f"w_t{i}")
                for kh in range(lo, hi):
                    w_t[kh] = (t, kh - lo)
                nc.sync.dma_start(out=t[:], in_=winT_rp[:, lo:hi, :])
            encT_sb[0] = _load_encT(0)
            encT_sb[1] = _load_encT(1)
            if not S1_BF16:
                nc.sync.dma_start(out=qTb_sb[:], in_=qTb_r[:])
            for g in range(4):
                nc.sync.dma_start(
                    out=w_out_sb[:, 4 * g : 4 * g + 4, :], in_=woutT_r[g]
                )
            for j in range(ENC_DMA_FROM, 2):
                _load_encN(j)
            for kh in range(KH):
                wt, wi = w_t[kh]
                for mg in range(KH):
                    nc.tensor.matmul(
                        qp_ps[mg][:],
                        wt[:, wi, ts(mg, P)],
                        qT_sb[:, kh, :],
                        start=(kh == 0),
                        stop=(kh == KH - 1),
                    )
                    if kh == KH - 1:
                        nc.vector.tensor_copy(qpT_sb[:, mg, :], qp_ps[mg][:])

        # ---- slot-loop state ----
        # tr-bank rotation for transpose staging (banks 6, 7)
        tr_state = [6]

        def tr_tile(name, dt=BF16, shape=(P, 4, P)):
            i = tr_state[0]
            tr_state[0] = 13 - i  # 6 <-> 7
            return ps.tile(list(shape), dt, tag=PS_TAGS[i], name=name,
                           padded_shape=[P, 4, P])

        p_sb = [None] * NB
        pT_sb = [None] * NB
        rinv = [None] * NB
        c_sb = [None] * NB
        cT_sb = [None] * NB
        o_ps = [None] * NB

        def emit_encNtr(j):
            """encN[j] (natural [s,h] tiles) from encT[j] via PE transposes."""
            t = sb.tile([P, KS[j], H], BF16, tag="encN", bufs=3,
                        name=f"encN_sb{j}")
            encN_sb[j] = t
            for ks in range(KS[j]):
                cw = REM[j] if ks == KS[j] - 1 else P
                for half in range(2):
                    e_ps = tr_tile(f"encNtr{j}_{ks}_{half}")
                    for i in range(4):
                        ih = half * 4 + i
                        nc.tensor.transpose(
                            e_ps[0:cw, i, :],
                            encT_sb[j][:, ih, ks * P : ks * P + cw],
                            idr_sb[:],
                        )
                    nc.vector.tensor_copy(
                        t[0:cw, ks, half * 512 : half * 512 + 512],
                        e_ps[0:cw, :, :],
                    )

        def emit_s2(j):
            """score(j) + additive length mask into sm bank (j%2)."""
            Ln = slot_lens[j]
            sc = ps_tile(j % 2, f"score{j}", shape=(P, Ln))
            for kh in range(KH):
                nc.tensor.matmul(
                    sc[:],
                    qpT_sb[:, kh, ts(j, T)],
                    encT_sb[j][:, kh, :],
                    start=(kh == 0),
                    stop=False,
                )
            nc.tensor.matmul(
                sc[:], ones_sb[:], mb_sb[:, j, 0:Ln], start=False, stop=True
            )
            return sc

        def emit_softmax(j, sc):
            Ln = slot_lens[j]
            negmax = sb.tile([P, 1], F32, tag="negmax", bufs=2, name="negmax")
            nc.vector.reduce_max(negmax[:], sc[:], axis=X.X, negate=True)
            p_sb[j] = sb.tile([P, KS[j] * P], BF16, tag="p", bufs=2,
                              name=f"p{j}")
            rowsum = sb.tile([P, 1], F32, tag="rowsum", bufs=2, name="rowsum")
            nc.scalar.activation(
                p_sb[j][:, 0:Ln], sc[:], AF.Exp, bias=negmax[:],
                accum_out=rowsum[:],
            )
            rinvb = sb.tile([P, 1], F32, tag="rinv", bufs=2, name=f"rinv{j}")
            nc.vector.reciprocal(rinvb[:], rowsum[:])
            # diag(1/rowsum): used as the cT-transpose "identity" so the
            # softmax normalization rides along for free.
            rinv[j] = sb.tile([P, P], BF16, tag="diag", bufs=2,
                              name=f"diag{j}")
            nc.vector.tensor_scalar_mul(rinv[j][:], idr_sb[:], rinvb[:])


        def emit_prefix(j):
            """S4 query-half: [q] @ W_out_q into a-banks (2, 3); no stop."""
            o_ps[j] = []
            for nh in range(NHALF):
                o = ps_tile(2 + nh, f"o_ps{j}_{nh}")
                o_ps[j].append(o)
                for kk in range(KH):
                    nc.tensor.matmul(
                        o[:],
                        qTb_sb[:, kk, ts(j, T)],
                        w_out_sb[:, kk, ts(nh, 512)],
                        start=(kk == 0),
                        stop=False,
                    )

        def emit_pT(j):
            """p -> pT chunks via PE transposes (one staging tile).  Reads
            full 128-chunks of the padded p tile; garbage columns land in
            partitions >= REM which S3 never reads."""
            pT_ps = tr_tile(f"pTtr{j}")
            for ks in range(KS[j]):
                nc.tensor.transpose(
                    pT_ps[:, ks, :], p_sb[j][:, ks * P : (ks + 1) * P],
                    idr_sb[:],
                )
            pT_sb[j] = sb.tile([P, KS[j], P], BF16, tag="pT", bufs=2,
                               name=f"pT{j}")
            nc.vector.tensor_copy(pT_sb[j][:], pT_ps[:, 0 : KS[j], :])

        def emit_s3(j):
            """c~ = pT^T @ encN into c banks (4, 5); plain-copy evict (the
            1/rowsum normalization is folded into the cT transpose)."""
            c_sb[j] = sb.tile([P, H], BF16, tag="c", bufs=2, name=f"c{j}")
            for nh in range(NHALF):
                c_ps = ps_tile(4 + nh, f"c_ps{j}_{nh}")
                for ks in range(KS[j]):
                    cw = REM[j] if ks == KS[j] - 1 else P
                    nc.tensor.matmul(
                        c_ps[:],
                        pT_sb[j][0:cw, ks, :],
                        encN_sb[j][0:cw, ks, ts(nh, 512)],
                        start=(ks == 0),
                        stop=(ks == KS[j] - 1),
                    )
                nc.vector.tensor_copy(c_sb[j][:, ts(nh, 512)], c_ps[:])

        def emit_cT(j):
            """c~ -> cT via regular matmuls against diag(1/rowsum): the PE
            transpose mode ignores its rhs, but c~^T @ diag(rinv) as a plain
            matmul transposes AND normalizes in one pass."""
            cT_sb[j] = sb.tile([P, KH, P], BF16, tag="cT", bufs=2,
                               name=f"cT{j}")
            for half in range(2):
                cT_ps = tr_tile(f"cTtr{j}_{half}", dt=F32)
                for i in range(4):
                    nc.tensor.matmul(
                        cT_ps[:, i, :], c_sb[j][:, ts(half * 4 + i, P)],
                        rinv[j][:], start=True, stop=True,
                    )
                nc.vector.tensor_copy(
                    cT_sb[j][:, half * 4 : half * 4 + 4, :], cT_ps[:]
                )

        def emit_suffix(j):
            """S4 context-half + bias; tanh-evict and store per nh half."""
            out_sb = sb.tile([P, H], F32, tag="out", bufs=2, name=f"out{j}")
            for nh in range(NHALF):
                nsl = ts(nh, 512)
                for kk in range(KH):
                    nc.tensor.matmul(
                        o_ps[j][nh][:],
                        cT_sb[j][:, kk, :],
                        w_out_sb[:, KH + kk, nsl],
                        start=False,
                        stop=False,
                    )
                nc.tensor.matmul(
                    o_ps[j][nh][:], ones_sb[:], bias_sb[:, nsl],
                    start=False, stop=True,
                )
                nc.scalar.activation(out_sb[:, nsl], o_ps[j][nh][:], AF.Tanh)
                nc.scalar.dma_start(out=out_d[j][:, nsl], in_=out_sb[:, nsl])

        # ---- prologue: slot 0 head; encNtr(0) + prefix(0) cover the
        # softmax(0) latency (S2(0) needs only encT0, so it goes first).
        with nc.named_scope("b0h"):
            sc0 = emit_s2(0)
            emit_softmax(0, sc0)
            if ENC_DMA_FROM > 0:
                emit_encNtr(0)
            else:
                _load_encN(0)
            emit_pT(0)
            emit_prefix(0)

        # ---- software-pipelined slot loop.  pT(b+1) sits after suffix(b)
        # so its exp() input is long done and its eviction hides under
        # prefix(b+1); S3(b+1) then starts the next iteration stall-free.
        for b in range(NB):
            scope = nc.named_scope(f"b{b}")
            scope.__enter__()
            emit_s3(b)
            emit_cT(b)
            if b + 1 < NB:
                if b + 2 < NB:
                    encT_sb[b + 2] = _load_encT(b + 2)
                    if b + 2 >= ENC_DMA_FROM:
                        _load_encN(b + 2)
                if b + 1 < ENC_DMA_FROM:
                    emit_encNtr(b + 1)
                sc = emit_s2(b + 1)
                emit_softmax(b + 1, sc)
            emit_suffix(b)
            if b + 1 < NB:
                emit_pT(b + 1)
                emit_prefix(b + 1)
            scope.__exit__(None, None, None)


def build_nc(slot_lens=(S,) * NB):
    # Bacc (not raw Bass): its lowering splits multi-sem waits and moves
    # matmul waits onto ldweights, which TRN2 codegen requires.
    nc = bacc.Bacc("TRN2", target_bir_lowering=False, debug=False)
    with tile.TileContext(nc) as tc:
        _emit(nc, tc, slot_lens)
    nc.compile()
    return nc


_NC_CACHE = {}


def _get_nc(slot_lens):
    key = (S1_DT, PAD, ENC_DMA_FROM, slot_lens)
    if key not in _NC_CACHE:
        _NC_CACHE[key] = build_nc(slot_lens)
    return _NC_CACHE[key]


def make_in_maps(query, encoder_outputs, src_lengths, W_in, W_out, b_out):
    """Host-side sharding + layout prep.

    Returns (in_maps, order, slot_lens): one input map per core; order maps
    (slot j, core c) -> original batch index order[j*NCORES + c].
    """
    query = np.asarray(query, dtype=np.float32)
    enc = np.asarray(encoder_outputs, dtype=np.float32)
    lens = np.asarray(src_lengths, dtype=np.int32)
    order, slot_lens = _slot_plan(lens)

    w_inT = np.ascontiguousarray(np.asarray(W_in, dtype=np.float32).T).astype(NP_S1)
    w_outT = np.ascontiguousarray(np.asarray(W_out, dtype=np.float32).T).astype(NP_BF16)
    bias = np.asarray(b_out, dtype=np.float32).astype(NP_BF16)
    ones = np.ones((P,), dtype=NP_BF16)

    in_maps = []
    for c in range(NCORES):
        idx = [int(order[j * NCORES + c]) for j in range(NB)]
        q_c = query[idx]                      # [NB, T, H] in slot order
        qT = np.ascontiguousarray(q_c.transpose(2, 0, 1)).reshape(H, TB)
        maskbias = np.where(
            np.arange(S, dtype=np.int64)[None, :]
            < lens[idx][:, None].astype(np.int64),
            np.float32(0.0),
            NEG,
        ).astype(NP_BF16)
        im = {
            "qT": qT.astype(NP_S1),
            **({} if S1_BF16 else {"qTb": qT.astype(NP_BF16)}),
            "winT": w_inT,
            "woutT": w_outT,
            "maskbias": maskbias,
            "bias": bias,
            "ones": ones,
        }
        for j in range(NB):
            Ln = slot_lens[j]
            e_b = enc[idx[j], :Ln, :]         # [Ln, H]
            im[f"encT{j}"] = np.ascontiguousarray(e_b.T).astype(NP_BF16)
            if j >= ENC_DMA_FROM:
                im[f"encN{j}"] = np.ascontiguousarray(e_b).astype(NP_BF16)
        in_maps.append(im)
    return in_maps, order, slot_lens


def run(query, encoder_outputs, src_lengths, W_in, W_out, b_out, **spmd_kwargs):
    in_maps, order, slot_lens = make_in_maps(
        query, encoder_outputs, src_lengths, W_in, W_out, b_out
    )
    res = run_bass_kernel_spmd(
        _get_nc(slot_lens), in_maps, list(range(NCORES)), **spmd_kwargs
    )
    out = np.empty((B, T, H), dtype=np.float32)
    for c in range(NCORES):
        core_out = res.results[c]["out"]      # [NB, T, H] in slot order
        for j in range(NB):
            out[int(order[j * NCORES + c])] = core_out[j]
    return out, res


def kernel(query, encoder_outputs, src_lengths, W_in, W_out, b_out):
    out, _ = run(query, encoder_outputs, src_lengths, W_in, W_out, b_out)
    return out
